# revision 1
# baseline (speedup 1.0000x reference)
"""Causal self-attention (B=2, S=4096, D=768, H=12) on 8 Trainium2 NeuronCores.

Sharding: data + head parallel. Core c handles batch c//4 and the 3 heads
starting at (c%4)*3. Each core computes the qkv projection for its heads,
causal attention, and a partial output projection (its heads' rows of w_out);
the host sums the 4 partial outputs per batch.

Device design notes:
 - x arrives pre-transposed (xT [768, 4096]) so the contraction dim lands on
   SBUF partitions for every projection matmul.
 - q, k are produced transposed ([hd, S]); scores are computed transposed
   ([sk, sq]) so the PV matmul consumes exp(scores) directly as the moving
   operand; a ones-column appended to v yields softmax denominators for free
   in the same matmul.
 - All matmuls run in bf16 (fp32 PSUM accumulate).
 - exp runs on ScalarE with the 1/sqrt(hd) scale fused into the activation
   affine; no max subtraction (scores are O(5) here, exp is safe in fp32).
 - Causal masking: only the 4 diagonal-chunk patterns need masking, applied
   as a GPSIMD affine_select (predicated fill) on exp(scores).
 - Softmax division: reciprocal_approx_fast (~51 ULP) on the denominator row,
   GPSIMD partition-broadcast, one DVE multiply.
 - All persistent activations are split into per-512-chunk tiles so the Tile
   scheduler can overlap projection, attention, and output phases.
"""

import numpy as np

try:
    import concourse.bass as bass  # noqa: F401
except ImportError:
    import sys
    sys.path.insert(0, "/opt/trn_rl_repo")

import concourse.bass as bass
import concourse.tile as tile
from concourse import bacc, mybir
from concourse.bass_utils import run_bass_kernel_spmd

F32 = mybir.dt.float32
F32R = mybir.dt.float32r
BF16 = mybir.dt.bfloat16
N_CORES = 8
B, S, D, H, HD = 2, 4096, 768, 12, 64
HPC = 3            # heads per core
SC = 512           # sequence chunk (free dim of most matmuls)
NSC = S // SC      # 8
KC = 128           # contraction chunk
NKC = D // KC      # 6
NQB = S // SC      # query blocks of 512
CPB = SC // KC     # key chunks per query block (4)
VW = HPC * (HD + 1)  # 195 v columns per key-chunk: [64 v | 1] x 3 heads

_CACHE = {}


def _emit(nc, tc, ins, out_ap):
    xT, wqk, wv, wo = ins
    MM = nc.tensor.matmul
    GE = mybir.AluOpType.is_ge

    constp = tc.alloc_tile_pool(name="const", bufs=1)
    xtp = tc.alloc_tile_pool(name="xt", bufs=12)
    qTp = tc.alloc_tile_pool(name="qTp", bufs=8)
    kTp = tc.alloc_tile_pool(name="kTp", bufs=8)
    q2p = tc.alloc_tile_pool(name="q2p", bufs=8)
    vp = tc.alloc_tile_pool(name="vp", bufs=8)
    ctxp = tc.alloc_tile_pool(name="ctx", bufs=24)
    expp = tc.alloc_tile_pool(name="exp", bufs=6)
    smp = tc.alloc_tile_pool(name="sm", bufs=3)
    ostp = tc.alloc_tile_pool(name="ost", bufs=3)
    psA = tc.alloc_tile_pool(name="psA", bufs=3, space="PSUM")   # shared ring
    psB = tc.alloc_tile_pool(name="psB", bufs=2, space="PSUM")   # ctx accumulators

    # ---- constants ----
    wqk_sb = constp.tile([128, NKC * 384], BF16, tag="wqk")
    for k in range(NKC):
        nc.sync.dma_start(wqk_sb[:, k * 384:(k + 1) * 384], wqk[k * 128:(k + 1) * 128, :])
    wv_sb = constp.tile([128, NKC * 256], BF16, tag="wv")
    for k in range(NKC):
        nc.sync.dma_start(wv_sb[:, k * 256:(k + 1) * 256], wv[k * 128:(k + 1) * 128, :])
    woAB_sb = constp.tile([128, 768], BF16, tag="woAB")
    nc.sync.dma_start(woAB_sb[:], wo[0:128, :])
    woC_sb = constp.tile([64, 768], BF16, tag="woC")
    nc.sync.dma_start(woC_sb[:], wo[128:192, :])

    # persistent activations, one tile per 512-wide sequence chunk.
    # matmul needs lhsT/rhs on the same partitions: head A uses rows 0:64 and
    # head B rows 64:128 of qT/kT (row-tiled concurrent matmuls).  head C's
    # q^T is duplicated on both row halves of q2k2 cols 0:512; its k^T chunks
    # sit on alternating row halves in cols 512:768, so consecutive head-C QK
    # matmuls hit distinct PE row groups (concurrent, weight loads hidden).
    qT = [qTp.tile([128, SC], BF16, tag="qT", name=f"qT{i}") for i in range(NSC)]
    kT = [kTp.tile([128, SC], BF16, tag="kT", name=f"kT{i}") for i in range(NSC)]
    q2k2 = [q2p.tile([128, SC + 256], BF16, tag="qk2", name=f"q2k2_{i}") for i in range(NSC)]
    vt = [vp.tile([128, CPB * VW], BF16, tag="v", name=f"vt{i}") for i in range(NSC)]
    # heads A,B stacked in one tile (rows 0:64 / 64:128) -> K=128 out-proj
    ctxAB = [ctxp.tile([128, SC], BF16, tag="ctxAB", name=f"ctxAB{i}")
             for i in range(NQB)]
    ctxC = [ctxp.tile([64, SC], BF16, tag="ctxC", name=f"ctxC{i}")
            for i in range(NQB)]

    ones_st = smp.tile([128, CPB], F32, tag="ones")
    nc.vector.memset(ones_st[:], 1.0)
    for i in range(NSC):
        v_r = vt[i][:].rearrange("p (c h e) -> p c h e", h=HPC, e=HD + 1)
        for h in range(HPC):
            nc.vector.tensor_copy(v_r[:, :, h, HD], ones_st[:])

    # ---- emission thunks -------------------------------------------------
    # PE executes its queue in emission order, so projection / attention /
    # output work is split into ~1-PSUM-bank pieces and woven together to
    # keep TensorE dense and ScalarE (exp) never starved.
    xts = {}

    def proj_pieces(sc):
        def dma_piece():
            xts[sc] = []
            for k in range(NKC):
                xt = xtp.tile([128, SC], BF16, tag="xt", name=f"xt{k}_{sc}")
                nc.sync.dma_start(xt[:], xT[k * 128:(k + 1) * 128, sc * SC:(sc + 1) * SC])
                xts[sc].append(xt)

        def m_piece(m):
            def f():
                ps = psA.tile([128, 2 * SC], F32, tag="sg", name=f"psqk{sc}_{m}")
                for k in range(NKC):
                    MM(ps[:, 0:SC], wqk_sb[:, k * 384 + m * 128: k * 384 + (m + 1) * 128],
                       xts[sc][k][:], start=(k == 0), stop=(k == NKC - 1))
                if m == 0:
                    nc.vector.tensor_copy(qT[sc][:], ps[:, 0:SC])
                elif m == 2:
                    nc.vector.tensor_copy(kT[sc][:], ps[:, 0:SC])
                else:
                    # qC duplicated on both row halves; kC chunks alternate halves
                    nc.vector.tensor_copy(q2k2[sc][0:64, 0:SC], ps[0:64, 0:SC])
                    nc.vector.tensor_copy(q2k2[sc][64:128, 0:SC], ps[0:64, 0:SC])
                    for j in range(CPB):
                        rh = (j % 2) * 64
                        nc.vector.tensor_copy(
                            q2k2[sc][rh:rh + 64, SC + (j // 2) * 128: SC + (j // 2 + 1) * 128],
                            ps[64:128, j * 128:(j + 1) * 128])
            return f

        def v_piece(jp):
            def f():
                v_r = vt[sc][:].rearrange("p (c h e) -> p c h e", h=HPC, e=HD + 1)
                pv = psA.tile([128, 2 * SC], F32, tag="sg", name=f"psv{sc}_{jp}")
                for j in (2 * jp, 2 * jp + 1):
                    for k in range(NKC):
                        MM(pv[:, (j % 2) * 256:(j % 2 + 1) * 256],
                           xts[sc][k][:, j * 128:(j + 1) * 128],
                           wv_sb[:, k * 256:(k + 1) * 256],
                           start=(k == 0), stop=(k == NKC - 1))
                for j in (2 * jp, 2 * jp + 1):
                    nc.vector.tensor_copy(
                        v_r[:, j, :, 0:HD],
                        pv[:, (j % 2) * 256:(j % 2) * 256 + HPC * HD]
                        .rearrange("p (h e) -> p h e", e=HD))
            return f

        return [dma_piece, m_piece(0), m_piece(1), m_piece(2), v_piece(0), v_piece(1)]

    def outproj_pieces(sc2):
        def p1():
            ost = ostp.tile([128, 768], F32, tag="ost", name=f"ost{sc2}")
            po = psA.tile([128, 2 * SC], F32, tag="sg", name=f"po{sc2}")
            csl = slice((sc2 % CPB) * 128, (sc2 % CPB + 1) * 128)
            MM(po[:, 0:512], ctxAB[sc2 // CPB][:, csl], woAB_sb[:, 0:512],
               start=True, stop=False)
            MM(po[:, 0:512], ctxC[sc2 // CPB][:, csl], woC_sb[:, 0:512],
               start=False, stop=True)
            MM(po[:, 512:768], ctxAB[sc2 // CPB][:, csl], woAB_sb[:, 512:768],
               start=True, stop=False)
            MM(po[:, 512:768], ctxC[sc2 // CPB][:, csl], woC_sb[:, 512:768],
               start=False, stop=True)
            nc.vector.tensor_copy(ost[:], po[:, 0:768])
            nc.sync.dma_start(out_ap[sc2 * 128:(sc2 + 1) * 128, :], ost[:])
        return [p1]

    def attention_thunks(sqb):
        nch = (sqb + 1) * CPB
        cps = {}

        def alloc(hlist):
            def f():
                for h in hlist:
                    cps[h] = psB.tile([128, SC], F32, tag="pb", name=f"cps{h}_{sqb}")
            return f

        pend = []  # (eg, grp): PV runs one group late so QK never stalls

        def emit_pv():
            eg, grp = pend.pop(0)
            for si, (h, ck) in enumerate(grp):
                osl = slice(si * SC, (si + 1) * SC)
                if ck >= nch - CPB:  # diagonal chunk: causal mask via fill
                    o = (ck - (nch - CPB)) * 128
                    nc.gpsimd.affine_select(
                        eg[:, osl], eg[:, osl], pattern=[[1, SC]],
                        compare_op=GE, fill=0.0, base=-o, channel_multiplier=-1)
                v_r = vt[ck // CPB][:].rearrange("p (c h e) -> p c h e",
                                                 h=HPC, e=HD + 1)
                MM(cps[h][0:HD + 1, :], v_r[:, ck % CPB, h, :], eg[:, osl],
                   start=(ck == 0), stop=(ck == nch - 1))

        def group(grp, gi):
            def f():
                sg = psA.tile([128, 2 * SC], F32, tag="sg", name=f"sg{sqb}_{gi}")
                eg = expp.tile([128, 2 * SC], BF16, tag="exp", name=f"eg{sqb}_{gi}")
                for si, (h, ck) in enumerate(grp):
                    osl = slice(si * SC, (si + 1) * SC)
                    if h < 2:
                        b0 = h * 64
                        MM(sg[:, osl],
                           kT[ck // CPB][b0:b0 + 64, (ck % CPB) * 128:(ck % CPB + 1) * 128],
                           qT[sqb][b0:b0 + 64, :], start=True, stop=True)
                    else:
                        rh = (ck % 2) * 64
                        MM(sg[:, osl],
                           q2k2[ck // CPB][rh:rh + 64,
                                           SC + (ck % CPB // 2) * 128: SC + (ck % CPB // 2 + 1) * 128],
                           q2k2[sqb][rh:rh + 64, 0:SC], start=True, stop=True)
                n = len(grp) * SC
                nc.scalar.activation(eg[:, 0:n], sg[:, 0:n],
                                     mybir.ActivationFunctionType.Exp, scale=0.125)
                pend.append((eg, grp))
                if len(pend) > 2:
                    emit_pv()
            return f

        def flush_pv():
            while pend:
                emit_pv()

        def norm(h):
            def f():
                # reciprocal_approx_fast is a bitwise-seed op and misreads
                # PSUM; bounce the denominator row through SBUF first
                dn = smp.tile([1, SC], F32, tag="dn", name=f"dn{h}_{sqb}")
                nc.vector.tensor_copy(dn[:], cps[h][HD:HD + 1, :])
                rec = smp.tile([1, SC], F32, tag="rec", name=f"rec{h}_{sqb}")
                nc.vector.reciprocal_approx_fast(rec[:], dn[:])
                bc = smp.tile([64, SC], F32, tag="bc", name=f"bc{h}_{sqb}")
                nc.gpsimd.partition_broadcast(bc[:], rec[:])
                if h == 0:
                    nc.vector.tensor_mul(ctxAB[sqb][0:64, :], cps[h][0:HD, :], bc[:])
                elif h == 1:
                    nc.vector.tensor_mul(ctxAB[sqb][64:128, :], cps[h][0:HD, :], bc[:])
                else:
                    nc.vector.tensor_mul(ctxC[sqb][:], cps[h][0:HD, :], bc[:])
            return f

        thunks = []
        for hlist in ((0, 1), (2,)):
            slots = [(h, ck) for ck in range(nch) for h in hlist]
            thunks.append(alloc(hlist))
            for gi, g0 in enumerate(range(0, len(slots), 2)):
                thunks.append(group(slots[g0:g0 + 2], f"{hlist[0]}_{gi}"))
            thunks.append(flush_pv)
            for h in hlist:
                thunks.append(norm(h))
        return thunks

    # ---- interleaved emission ----
    for piece in proj_pieces(0):
        piece()
    for sqb in range(NQB):
        groups = attention_thunks(sqb)
        extras = []
        if sqb + 1 < NSC:
            extras += proj_pieces(sqb + 1)
        if sqb >= 1:
            for j in range(CPB):
                extras += outproj_pieces((sqb - 1) * CPB + j)
        n, k = len(groups), len(extras)
        ei = 0
        for i, g in enumerate(groups):
            g()
            due = (i + 1) * k // n
            while ei < due:
                extras[ei]()
                ei += 1
    for j in range(CPB):
        for piece in outproj_pieces((NQB - 1) * CPB + j):
            piece()

    for p in (psB, psA, ostp, smp, expp, ctxp, vp, q2p, kTp, qTp, xtp, constp):
        p.release()


def _build():
    if "nc" in _CACHE:
        return _CACHE["nc"]
    nc = bacc.Bacc("TRN2", target_bir_lowering=False, debug=False, num_devices=N_CORES)
    xT = nc.dram_tensor("xT", [D, S], BF16, kind="ExternalInput").ap()
    wqk = nc.dram_tensor("wqk", [D, 384], BF16, kind="ExternalInput").ap()
    wv = nc.dram_tensor("wv", [D, 256], BF16, kind="ExternalInput").ap()
    wo = nc.dram_tensor("wo", [HPC * HD, D], BF16, kind="ExternalInput").ap()
    out = nc.dram_tensor("out", [S, D], F32, kind="ExternalOutput").ap()
    with tile.TileContext(nc) as tc:
        _emit(nc, tc, (xT, wqk, wv, wo), out)
    nc.compile()
    _CACHE["nc"] = nc
    return nc


def _in_maps(x, w_qkv, w_out):
    import ml_dtypes
    xTs = [np.ascontiguousarray(x[b].T).astype(ml_dtypes.bfloat16) for b in range(B)]
    maps = []
    for c in range(N_CORES):
        b = c // 4
        h0 = (c % 4) * HPC
        cols = lambda base, h: w_qkv[:, base + (h0 + h) * HD: base + (h0 + h + 1) * HD]
        wqk = np.ascontiguousarray(np.concatenate(
            [cols(0, 0), cols(0, 1),            # m0: qA | qB
             cols(0, 2), cols(D, 2),            # m1: qC | kC
             cols(D, 0), cols(D, 1)], axis=1)).astype(ml_dtypes.bfloat16)
        wv = np.ascontiguousarray(np.concatenate(
            [cols(2 * D, 0), cols(2 * D, 1), cols(2 * D, 2),
             np.zeros((D, 64), np.float32)], axis=1)).astype(ml_dtypes.bfloat16)
        wo = np.ascontiguousarray(
            w_out[h0 * HD:(h0 + HPC) * HD, :]).astype(ml_dtypes.bfloat16)
        maps.append({"xT": xTs[b], "wqk": wqk, "wv": wv, "wo": wo})
    return maps


def run_sharded(x, w_qkv, w_out, **spmd_kwargs):
    nc = _build()
    res = run_bass_kernel_spmd(nc, _in_maps(x, w_qkv, w_out),
                               list(range(N_CORES)), **spmd_kwargs)
    outs = [res.results[c]["out"] for c in range(N_CORES)]
    y = np.empty((B, S, D), np.float32)
    for b in range(B):
        y[b] = outs[4 * b] + outs[4 * b + 1] + outs[4 * b + 2] + outs[4 * b + 3]
    return y, res


def kernel(x, w_qkv, w_out):
    x = np.asarray(x, dtype=np.float32)
    w_qkv = np.asarray(w_qkv, dtype=np.float32)
    w_out = np.asarray(w_out, dtype=np.float32)
    y, _ = run_sharded(x, w_qkv, w_out)
    return y



# revision 9
# speedup vs baseline: 1.0192x; 1.0192x over previous
"""Causal self-attention (B=2, S=4096, D=768, H=12) on 8 Trainium2 NeuronCores.

Sharding: data + head parallel. Core c handles batch c//4 and the 3 heads
starting at (c%4)*3. Each core computes the qkv projection for its heads,
causal attention, and a partial output projection (its heads' rows of w_out);
the host sums the 4 partial outputs per batch.

Device design notes:
 - x arrives pre-transposed (xT [768, 4096]) so the contraction dim lands on
   SBUF partitions for every projection matmul.
 - q, k are produced transposed ([hd, S]); scores are computed transposed
   ([sk, sq]) so the PV matmul consumes exp(scores) directly as the moving
   operand; a ones-column appended to v yields softmax denominators for free
   in the same matmul.
 - All matmuls run in bf16 (fp32 PSUM accumulate).
 - exp runs on ScalarE with the 1/sqrt(hd) scale fused into the activation
   affine; no max subtraction (scores are O(5) here, exp is safe in fp32).
 - Causal masking: only the 4 diagonal-chunk patterns need masking, applied
   as a GPSIMD affine_select (predicated fill) on exp(scores).
 - Softmax division: reciprocal_approx_fast (~51 ULP) on the denominator row,
   GPSIMD partition-broadcast, one DVE multiply.
 - All persistent activations are split into per-512-chunk tiles so the Tile
   scheduler can overlap projection, attention, and output phases.
"""

import numpy as np

try:
    import concourse.bass as bass  # noqa: F401
except ImportError:
    import sys
    sys.path.insert(0, "/opt/trn_rl_repo")

import concourse.bass as bass
import concourse.tile as tile
from concourse import bacc, mybir
from concourse.bass_utils import run_bass_kernel_spmd

F32 = mybir.dt.float32
F32R = mybir.dt.float32r
BF16 = mybir.dt.bfloat16
N_CORES = 8
B, S, D, H, HD = 2, 4096, 768, 12, 64
HPC = 3            # heads per core
SC = 512           # sequence chunk (free dim of most matmuls)
NSC = S // SC      # 8
KC = 128           # contraction chunk
NKC = D // KC      # 6
NQB = S // SC      # query blocks of 512
CPB = SC // KC     # key chunks per query block (4)
VW = HPC * (HD + 1)  # 195 v columns per key-chunk: [64 v | 1] x 3 heads
VTW = CPB * VW + 68  # vt tile width, padded so every 128-wide PV window fits

_CACHE = {}


def _emit(nc, tc, ins, out_ap):
    xT, wqk, wv, wo = ins
    MM = nc.tensor.matmul
    GE = mybir.AluOpType.is_ge

    constp = tc.alloc_tile_pool(name="const", bufs=1)
    xtp = tc.alloc_tile_pool(name="xt", bufs=12)
    qTp = tc.alloc_tile_pool(name="qTp", bufs=8)
    kTp = tc.alloc_tile_pool(name="kTp", bufs=8)
    q2p = tc.alloc_tile_pool(name="q2p", bufs=8)
    vp = tc.alloc_tile_pool(name="vp", bufs=8)
    ctxp = tc.alloc_tile_pool(name="ctx", bufs=24)
    expp = tc.alloc_tile_pool(name="exp", bufs=6)
    smp = tc.alloc_tile_pool(name="sm", bufs=3)
    ostp = tc.alloc_tile_pool(name="ost", bufs=3)
    psA = tc.alloc_tile_pool(name="psA", bufs=3, space="PSUM")   # shared ring
    psB = tc.alloc_tile_pool(name="psB", bufs=2, space="PSUM")   # ctx accumulators

    # ---- constants ----
    wqk_sb = constp.tile([128, NKC * 384], BF16, tag="wqk")
    for k in range(NKC):
        nc.sync.dma_start(wqk_sb[:, k * 384:(k + 1) * 384], wqk[k * 128:(k + 1) * 128, :])
    wv_sb = constp.tile([128, NKC * 192], BF16, tag="wv")
    for k in range(NKC):
        nc.sync.dma_start(wv_sb[:, k * 192:(k + 1) * 192], wv[k * 128:(k + 1) * 128, :])
    woAB_sb = constp.tile([128, 768], BF16, tag="woAB")
    nc.sync.dma_start(woAB_sb[:], wo[0:128, :])
    woC_sb = constp.tile([64, 768], BF16, tag="woC")
    nc.sync.dma_start(woC_sb[:], wo[128:192, :])

    # persistent activations, one tile per 512-wide sequence chunk.
    # matmul needs lhsT/rhs on the same partitions: head A uses rows 0:64 and
    # head B rows 64:128 of qT/kT (row-tiled concurrent matmuls).  head C's
    # q^T is duplicated on both row halves of q2k2 cols 0:512; its k^T chunks
    # sit on alternating row halves in cols 512:768, so consecutive head-C QK
    # matmuls hit distinct PE row groups (concurrent, weight loads hidden).
    qT = [qTp.tile([128, SC], BF16, tag="qT", name=f"qT{i}") for i in range(NSC)]
    kT = [kTp.tile([128, SC], BF16, tag="kT", name=f"kT{i}") for i in range(NSC)]
    q2k2 = [q2p.tile([128, SC + 256], BF16, tag="qk2", name=f"q2k2_{i}") for i in range(NSC)]
    vt = [vp.tile([128, VTW], BF16, tag="v", name=f"vt{i}") for i in range(NSC)]
    # heads A,B stacked in one tile (rows 0:64 / 64:128) -> K=128 out-proj
    ctxAB = [ctxp.tile([128, SC], BF16, tag="ctxAB", name=f"ctxAB{i}")
             for i in range(NQB)]
    ctxC = [ctxp.tile([64, SC], BF16, tag="ctxC", name=f"ctxC{i}")
            for i in range(NQB)]

    ones_st = smp.tile([128, CPB], F32, tag="ones")
    nc.vector.memset(ones_st[:], 1.0)
    for i in range(NSC):
        v_r = vt[i][:, 0:CPB * VW].rearrange("p (c h e) -> p c h e", h=HPC, e=HD + 1)
        for h in range(HPC):
            nc.vector.tensor_copy(v_r[:, :, h, HD], ones_st[:])
        nc.vector.memset(vt[i][:, CPB * VW:VTW], 0.0)

    # ---- emission thunks -------------------------------------------------
    # PE executes its queue in emission order, so projection / attention /
    # output work is split into ~1-PSUM-bank pieces and woven together to
    # keep TensorE dense and ScalarE (exp) never starved.
    xts = {}

    def proj_pieces(sc):
        def dma_piece():
            xts[sc] = []
            for k in range(NKC):
                xt = xtp.tile([128, SC], BF16, tag="xt", name=f"xt{k}_{sc}")
                nc.sync.dma_start(xt[:], xT[k * 128:(k + 1) * 128, sc * SC:(sc + 1) * SC])
                xts[sc].append(xt)

        def m_piece(m):
            def f():
                ps = psA.tile([128, 2 * SC], F32, tag="sg", name=f"psqk{sc}_{m}")
                for k in range(NKC):
                    MM(ps[:, 0:SC], wqk_sb[:, k * 384 + m * 128: k * 384 + (m + 1) * 128],
                       xts[sc][k][:], start=(k == 0), stop=(k == NKC - 1))
                if m == 0:
                    nc.vector.tensor_copy(qT[sc][:], ps[:, 0:SC])
                elif m == 2:
                    nc.vector.tensor_copy(kT[sc][:], ps[:, 0:SC])
                else:
                    # qC duplicated on both row halves; kC chunks alternate halves
                    nc.vector.tensor_copy(q2k2[sc][0:64, 0:SC], ps[0:64, 0:SC])
                    nc.vector.tensor_copy(q2k2[sc][64:128, 0:SC], ps[0:64, 0:SC])
                    for j in range(CPB):
                        rh = (j % 2) * 64
                        nc.vector.tensor_copy(
                            q2k2[sc][rh:rh + 64, SC + (j // 2) * 128: SC + (j // 2 + 1) * 128],
                            ps[64:128, j * 128:(j + 1) * 128])
            return f

        def v_piece(jp):
            def f():
                v_r = vt[sc][:, 0:CPB * VW].rearrange("p (c h e) -> p c h e",
                                                      h=HPC, e=HD + 1)
                pv = psA.tile([128, 2 * SC], F32, tag="sg", name=f"psv{sc}_{jp}")
                for j in (2 * jp, 2 * jp + 1):
                    for k in range(NKC):
                        MM(pv[:, (j % 2) * 512:(j % 2) * 512 + 192],
                           xts[sc][k][:, j * 128:(j + 1) * 128],
                           wv_sb[:, k * 192:(k + 1) * 192],
                           start=(k == 0), stop=(k == NKC - 1))
                for j in (2 * jp, 2 * jp + 1):
                    nc.vector.tensor_copy(
                        v_r[:, j, :, 0:HD],
                        pv[:, (j % 2) * 512:(j % 2) * 512 + HPC * HD]
                        .rearrange("p (h e) -> p h e", e=HD))
            return f

        return [dma_piece, m_piece(0), m_piece(1), m_piece(2), v_piece(0), v_piece(1)]

    def outproj_pieces(sc2):
        def p1():
            ost = ostp.tile([128, 768], F32, tag="ost", name=f"ost{sc2}")
            po = psA.tile([128, 2 * SC], F32, tag="sg", name=f"po{sc2}")
            csl = slice((sc2 % CPB) * 128, (sc2 % CPB + 1) * 128)
            MM(po[:, 0:512], ctxAB[sc2 // CPB][:, csl], woAB_sb[:, 0:512],
               start=True, stop=False)
            MM(po[:, 0:512], ctxC[sc2 // CPB][:, csl], woC_sb[:, 0:512],
               start=False, stop=True)
            MM(po[:, 512:768], ctxAB[sc2 // CPB][:, csl], woAB_sb[:, 512:768],
               start=True, stop=False)
            MM(po[:, 512:768], ctxC[sc2 // CPB][:, csl], woC_sb[:, 512:768],
               start=False, stop=True)
            nc.vector.tensor_copy(ost[:], po[:, 0:768])
            nc.sync.dma_start(out_ap[sc2 * 128:(sc2 + 1) * 128, :], ost[:])
        return [p1]

    def attention_thunks(sqb):
        nch = (sqb + 1) * CPB
        cps = {}

        def alloc(hlist):
            def f():
                for h in hlist:
                    cps[h] = psB.tile([128, SC], F32, tag="pb", name=f"cps{h}_{sqb}")
            return f

        pend = []  # (eg, grp): PV runs one group late so QK never stalls

        def emit_pv():
            eg, grp = pend.pop(0)
            for si, (h, ck) in enumerate(grp):
                osl = slice(si * SC, (si + 1) * SC)
                if ck >= nch - CPB:  # diagonal chunk: causal mask via fill
                    o = (ck - (nch - CPB)) * 128
                    nc.gpsimd.affine_select(
                        eg[:, osl], eg[:, osl], pattern=[[1, SC]],
                        compare_op=GE, fill=0.0, base=-o, channel_multiplier=-1)
                off = (ck % CPB) * VW + h * (HD + 1)
                MM(cps[h][:, :], vt[ck // CPB][:, off:off + 128], eg[:, osl],
                   start=(ck == 0), stop=(ck == nch - 1))

        def group(grp, gi):
            def f():
                sg = psA.tile([128, 2 * SC], F32, tag="sg", name=f"sg{sqb}_{gi}")
                eg = expp.tile([128, 2 * SC], BF16, tag="exp", name=f"eg{sqb}_{gi}")
                for si, (h, ck) in enumerate(grp):
                    osl = slice(si * SC, (si + 1) * SC)
                    if h < 2:
                        b0 = h * 64
                        MM(sg[:, osl],
                           kT[ck // CPB][b0:b0 + 64, (ck % CPB) * 128:(ck % CPB + 1) * 128],
                           qT[sqb][b0:b0 + 64, :], start=True, stop=True)
                    else:
                        rh = (ck % 2) * 64
                        MM(sg[:, osl],
                           q2k2[ck // CPB][rh:rh + 64,
                                           SC + (ck % CPB // 2) * 128: SC + (ck % CPB // 2 + 1) * 128],
                           q2k2[sqb][rh:rh + 64, 0:SC], start=True, stop=True)
                n = len(grp) * SC
                nc.scalar.activation(eg[:, 0:n], sg[:, 0:n],
                                     mybir.ActivationFunctionType.Exp, scale=0.125)
                pend.append((eg, grp))
                if len(pend) > 2:
                    emit_pv()
            return f

        def flush_pv():
            while pend:
                emit_pv()

        def norm(h):
            def f():
                # reciprocal_approx_fast is a bitwise-seed op and misreads
                # PSUM; bounce the denominator row through SBUF first
                dn = smp.tile([1, SC], F32, tag="dn", name=f"dn{h}_{sqb}")
                nc.vector.tensor_copy(dn[:], cps[h][HD:HD + 1, :])
                rec = smp.tile([1, SC], F32, tag="rec", name=f"rec{h}_{sqb}")
                nc.vector.reciprocal_approx_fast(rec[:], dn[:])
                bc = smp.tile([64, SC], F32, tag="bc", name=f"bc{h}_{sqb}")
                nc.gpsimd.partition_broadcast(bc[:], rec[:])
                if h == 0:
                    nc.vector.tensor_mul(ctxAB[sqb][0:64, :], cps[h][0:HD, :], bc[:])
                elif h == 1:
                    nc.vector.tensor_mul(ctxAB[sqb][64:128, :], cps[h][0:HD, :], bc[:])
                else:
                    nc.vector.tensor_mul(ctxC[sqb][:], cps[h][0:HD, :], bc[:])
            return f

        thunks = []
        for hlist in ((0, 1), (2,)):
            slots = [(h, ck) for ck in range(nch) for h in hlist]
            thunks.append(alloc(hlist))
            for gi, g0 in enumerate(range(0, len(slots), 2)):
                thunks.append(group(slots[g0:g0 + 2], f"{hlist[0]}_{gi}"))
            thunks.append(flush_pv)
            for h in hlist:
                thunks.append(norm(h))
        return thunks

    # ---- interleaved emission ----
    for piece in proj_pieces(0):
        piece()
    for sqb in range(NQB):
        groups = attention_thunks(sqb)
        extras = []
        if sqb + 1 < NSC:
            extras += proj_pieces(sqb + 1)
        if sqb >= 1:
            for j in range(CPB):
                extras += outproj_pieces((sqb - 1) * CPB + j)
        n, k = len(groups), len(extras)
        ei = 0
        for i, g in enumerate(groups):
            g()
            due = (i + 1) * k // n
            while ei < due:
                extras[ei]()
                ei += 1
    for j in range(CPB):
        for piece in outproj_pieces((NQB - 1) * CPB + j):
            piece()

    for p in (psB, psA, ostp, smp, expp, ctxp, vp, q2p, kTp, qTp, xtp, constp):
        p.release()


def _build():
    if "nc" in _CACHE:
        return _CACHE["nc"]
    nc = bacc.Bacc("TRN2", target_bir_lowering=False, debug=False, num_devices=N_CORES)
    xT = nc.dram_tensor("xT", [D, S], BF16, kind="ExternalInput").ap()
    wqk = nc.dram_tensor("wqk", [D, 384], BF16, kind="ExternalInput").ap()
    wv = nc.dram_tensor("wv", [D, 192], BF16, kind="ExternalInput").ap()
    wo = nc.dram_tensor("wo", [HPC * HD, D], BF16, kind="ExternalInput").ap()
    out = nc.dram_tensor("out", [S, D], F32, kind="ExternalOutput").ap()
    with tile.TileContext(nc) as tc:
        _emit(nc, tc, (xT, wqk, wv, wo), out)
    nc.compile()
    _CACHE["nc"] = nc
    return nc


def _in_maps(x, w_qkv, w_out):
    import ml_dtypes
    xTs = [np.ascontiguousarray(x[b].T).astype(ml_dtypes.bfloat16) for b in range(B)]
    maps = []
    for c in range(N_CORES):
        b = c // 4
        h0 = (c % 4) * HPC
        cols = lambda base, h: w_qkv[:, base + (h0 + h) * HD: base + (h0 + h + 1) * HD]
        wqk = np.ascontiguousarray(np.concatenate(
            [cols(0, 0), cols(0, 1),            # m0: qA | qB
             cols(0, 2), cols(D, 2),            # m1: qC | kC
             cols(D, 0), cols(D, 1)], axis=1)).astype(ml_dtypes.bfloat16)
        wv = np.ascontiguousarray(np.concatenate(
            [cols(2 * D, 0), cols(2 * D, 1), cols(2 * D, 2)],
            axis=1)).astype(ml_dtypes.bfloat16)
        wo = np.ascontiguousarray(
            w_out[h0 * HD:(h0 + HPC) * HD, :]).astype(ml_dtypes.bfloat16)
        maps.append({"xT": xTs[b], "wqk": wqk, "wv": wv, "wo": wo})
    return maps


def run_sharded(x, w_qkv, w_out, **spmd_kwargs):
    nc = _build()
    res = run_bass_kernel_spmd(nc, _in_maps(x, w_qkv, w_out),
                               list(range(N_CORES)), **spmd_kwargs)
    outs = [res.results[c]["out"] for c in range(N_CORES)]
    y = np.empty((B, S, D), np.float32)
    for b in range(B):
        y[b] = outs[4 * b] + outs[4 * b + 1] + outs[4 * b + 2] + outs[4 * b + 3]
    return y, res


def kernel(x, w_qkv, w_out):
    x = np.asarray(x, dtype=np.float32)
    w_qkv = np.asarray(w_qkv, dtype=np.float32)
    w_out = np.asarray(w_out, dtype=np.float32)
    y, _ = run_sharded(x, w_qkv, w_out)
    return y



# revision 16
# speedup vs baseline: 1.0897x; 1.0692x over previous
"""Causal self-attention (B=2, S=4096, D=768, H=12) on 8 Trainium2 NeuronCores.

Sharding: data + head parallel. Core c handles batch c//4 and the 3 heads
starting at (c%4)*3. Each core computes the qkv projection for its heads,
causal attention, and a partial output projection (its heads' rows of w_out);
the host sums the 4 partial outputs per batch.

Device design notes:
 - x arrives pre-transposed (xT [768, 4096]) so the contraction dim lands on
   SBUF partitions for every projection matmul.
 - q, k are produced transposed ([hd, S]); scores are computed transposed
   ([sk, sq]) so the PV matmul consumes exp(scores) directly as the moving
   operand; a ones-column appended to v yields softmax denominators for free
   in the same matmul.
 - All matmuls run in bf16 (fp32 PSUM accumulate).
 - exp runs on ScalarE with the 1/sqrt(hd) scale fused into the activation
   affine; no max subtraction (scores are O(5) here, exp is safe in fp32).
 - Causal masking: only the 4 diagonal-chunk patterns need masking, applied
   as a GPSIMD affine_select (predicated fill) on exp(scores).
 - Softmax division: reciprocal_approx_fast (~51 ULP) on the denominator row,
   GPSIMD partition-broadcast, one DVE multiply.
 - All persistent activations are split into per-512-chunk tiles so the Tile
   scheduler can overlap projection, attention, and output phases.
"""

import numpy as np

try:
    import concourse.bass as bass  # noqa: F401
except ImportError:
    import sys
    sys.path.insert(0, "/opt/trn_rl_repo")

import concourse.bass as bass
import concourse.tile as tile
from concourse import bacc, mybir
from concourse.bass_utils import run_bass_kernel_spmd

F32 = mybir.dt.float32
F32R = mybir.dt.float32r
BF16 = mybir.dt.bfloat16
N_CORES = 8
B, S, D, H, HD = 2, 4096, 768, 12, 64
HPC = 3            # heads per core
SC = 512           # sequence chunk (free dim of most matmuls)
NSC = S // SC      # 8
KC = 128           # contraction chunk
NKC = D // KC      # 6
NQB = S // SC      # query blocks of 512
CPB = SC // KC     # key chunks per query block (4)
VW = HPC * (HD + 1)  # 195 v columns per key-chunk: [64 v | 1] x 3 heads
VTW = CPB * VW + 68  # vt tile width, padded so every 128-wide PV window fits

_CACHE = {}


def _emit(nc, tc, ins, out_ap):
    xT, wqk, wv, wo = ins
    MM = nc.tensor.matmul
    GE = mybir.AluOpType.is_ge

    constp = tc.alloc_tile_pool(name="const", bufs=1)
    xtp = tc.alloc_tile_pool(name="xt", bufs=3)
    qTp = tc.alloc_tile_pool(name="qTp", bufs=8)
    kTp = tc.alloc_tile_pool(name="kTp", bufs=8)
    q2p = tc.alloc_tile_pool(name="q2p", bufs=8)
    vp = tc.alloc_tile_pool(name="vp", bufs=8)
    ctxp = tc.alloc_tile_pool(name="ctx", bufs=24)
    expp = tc.alloc_tile_pool(name="exp", bufs=6)
    smp = tc.alloc_tile_pool(name="sm", bufs=3)
    ostp = tc.alloc_tile_pool(name="ost", bufs=3)
    psA = tc.alloc_tile_pool(name="psA", bufs=3, space="PSUM")   # shared ring
    psB = tc.alloc_tile_pool(name="psB", bufs=2, space="PSUM")   # ctx accumulators

    # ---- first xt load goes out on sync ahead of everything; weights issue
    # from the Activation HWDGE queue (ScalarE is idle at startup) so the two
    # descriptor-gen streams run concurrently.
    xts = {}

    def dma_xt(sc):
        xt = xtp.tile([128, NKC * SC], BF16, tag="xt", name=f"xt{sc}")
        nc.sync.dma_start(
            xt[:].rearrange("p (c n) -> p c n", c=NKC),
            xT[:, sc * SC:(sc + 1) * SC].rearrange("(c p) n -> p c n", p=128))
        xts[sc] = xt

    dma_xt(0)

    wqk_sb = constp.tile([128, NKC * 384], BF16, tag="wqk")
    nc.scalar.dma_start(
        wqk_sb[:].rearrange("p (c m) -> p c m", c=NKC),
        wqk.rearrange("(c p) m -> p c m", p=128))
    wv_sb = constp.tile([128, NKC * 192], BF16, tag="wv")
    nc.scalar.dma_start(
        wv_sb[:].rearrange("p (c m) -> p c m", c=NKC),
        wv.rearrange("(c p) m -> p c m", p=128))
    woAB_sb = constp.tile([128, 768], BF16, tag="woAB")
    nc.scalar.dma_start(woAB_sb[:], wo[0:128, :])
    woC_sb = constp.tile([64, 768], BF16, tag="woC")
    nc.scalar.dma_start(woC_sb[:], wo[128:192, :])

    # persistent activations, one tile per 512-wide sequence chunk.
    # matmul needs lhsT/rhs on the same partitions: head A uses rows 0:64 and
    # head B rows 64:128 of qT/kT (row-tiled concurrent matmuls).  head C's
    # q^T is duplicated on both row halves of q2k2 cols 0:512; its k^T chunks
    # sit on alternating row halves in cols 512:768, so consecutive head-C QK
    # matmuls hit distinct PE row groups (concurrent, weight loads hidden).
    qT = [qTp.tile([128, SC], BF16, tag="qT", name=f"qT{i}") for i in range(NSC)]
    kT = [kTp.tile([128, SC], BF16, tag="kT", name=f"kT{i}") for i in range(NSC)]
    q2k2 = [q2p.tile([128, SC + 256], BF16, tag="qk2", name=f"q2k2_{i}") for i in range(NSC)]
    vt = [vp.tile([128, VTW], BF16, tag="v", name=f"vt{i}") for i in range(NSC)]
    # heads A,B stacked in one tile (rows 0:64 / 64:128) -> K=128 out-proj
    ctxAB = [ctxp.tile([128, SC], BF16, tag="ctxAB", name=f"ctxAB{i}")
             for i in range(NQB)]
    ctxC = [ctxp.tile([64, SC], BF16, tag="ctxC", name=f"ctxC{i}")
            for i in range(NQB)]

    ones_st = smp.tile([128, CPB], F32, tag="ones")
    nc.vector.memset(ones_st[:], 1.0)
    for i in range(NSC):
        v_r = vt[i][:, 0:CPB * VW].rearrange("p (c h e) -> p c h e", h=HPC, e=HD + 1)
        for h in range(HPC):
            nc.vector.tensor_copy(v_r[:, :, h, HD], ones_st[:])
        nc.vector.memset(vt[i][:, CPB * VW:VTW], 0.0)

    # ---- emission thunks -------------------------------------------------
    # PE executes its queue in emission order, so projection / attention /
    # output work is split into ~1-PSUM-bank pieces and woven together to
    # keep TensorE dense and ScalarE (exp) never starved.

    def proj_pieces(sc):
        def dma_piece():
            if sc not in xts:
                dma_xt(sc)

        def m_piece(m):
            def f():
                ps = psA.tile([128, 2 * SC], F32, tag="sg", name=f"psqk{sc}_{m}")
                for k in range(NKC):
                    MM(ps[:, 0:SC], wqk_sb[:, k * 384 + m * 128: k * 384 + (m + 1) * 128],
                       xts[sc][:, k * SC:(k + 1) * SC], start=(k == 0), stop=(k == NKC - 1))
                if m == 0:
                    nc.vector.tensor_copy(qT[sc][:], ps[:, 0:SC])
                elif m == 2:
                    nc.vector.tensor_copy(kT[sc][:], ps[:, 0:SC])
                else:
                    # qC duplicated on both row halves; kC chunks alternate halves
                    nc.vector.tensor_copy(q2k2[sc][0:64, 0:SC], ps[0:64, 0:SC])
                    nc.vector.tensor_copy(q2k2[sc][64:128, 0:SC], ps[0:64, 0:SC])
                    for j in range(CPB):
                        rh = (j % 2) * 64
                        nc.vector.tensor_copy(
                            q2k2[sc][rh:rh + 64, SC + (j // 2) * 128: SC + (j // 2 + 1) * 128],
                            ps[64:128, j * 128:(j + 1) * 128])
            return f

        def v_piece(jp):
            def f():
                v_r = vt[sc][:, 0:CPB * VW].rearrange("p (c h e) -> p c h e",
                                                      h=HPC, e=HD + 1)
                pv = psA.tile([128, 2 * SC], F32, tag="sg", name=f"psv{sc}_{jp}")
                for j in (2 * jp, 2 * jp + 1):
                    for k in range(NKC):
                        MM(pv[:, (j % 2) * 512:(j % 2) * 512 + 192],
                           xts[sc][:, k * SC + j * 128:k * SC + (j + 1) * 128],
                           wv_sb[:, k * 192:(k + 1) * 192],
                           start=(k == 0), stop=(k == NKC - 1))
                for j in (2 * jp, 2 * jp + 1):
                    nc.vector.tensor_copy(
                        v_r[:, j, :, 0:HD],
                        pv[:, (j % 2) * 512:(j % 2) * 512 + HPC * HD]
                        .rearrange("p (h e) -> p h e", e=HD))
            return f

        return [dma_piece, m_piece(0), m_piece(1), m_piece(2), v_piece(0), v_piece(1)]

    def outproj_pieces(sc2):
        def p1():
            ost = ostp.tile([128, 768], F32, tag="ost", name=f"ost{sc2}")
            po = psA.tile([128, 2 * SC], F32, tag="sg", name=f"po{sc2}")
            csl = slice((sc2 % CPB) * 128, (sc2 % CPB + 1) * 128)
            MM(po[:, 0:512], ctxAB[sc2 // CPB][:, csl], woAB_sb[:, 0:512],
               start=True, stop=False)
            MM(po[:, 0:512], ctxC[sc2 // CPB][:, csl], woC_sb[:, 0:512],
               start=False, stop=True)
            MM(po[:, 512:768], ctxAB[sc2 // CPB][:, csl], woAB_sb[:, 512:768],
               start=True, stop=False)
            MM(po[:, 512:768], ctxC[sc2 // CPB][:, csl], woC_sb[:, 512:768],
               start=False, stop=True)
            nc.vector.tensor_copy(ost[:], po[:, 0:768])
            nc.sync.dma_start(out_ap[sc2 * 128:(sc2 + 1) * 128, :], ost[:])
        return [p1]

    def attention_thunks(sqb):
        nch = (sqb + 1) * CPB
        cps = {}

        def alloc(hlist):
            def f():
                for h in hlist:
                    cps[h] = psB.tile([128, SC], F32, tag="pb", name=f"cps{h}_{sqb}")
            return f

        d0 = nch - CPB  # first diagonal chunk

        def width(ck):
            return SC - (ck - d0) * 128 if ck >= d0 else SC

        def qoff(ck):
            return (ck - d0) * 128 if ck >= d0 else 0

        pend = []  # (eg, grp): PV runs one group late so QK never stalls

        def emit_pv():
            eg, grp = pend.pop(0)
            for h, ck, off, w in grp:
                if ck >= d0:  # diagonal chunk: causal mask on leading 128 cols
                    nc.gpsimd.affine_select(
                        eg[:, off:off + 128], eg[:, off:off + 128],
                        pattern=[[1, 128]], compare_op=GE, fill=0.0,
                        base=0, channel_multiplier=-1)
                vo = (ck % CPB) * VW + h * (HD + 1)
                MM(cps[h][:, qoff(ck):qoff(ck) + w],
                   vt[ck // CPB][:, vo:vo + 128], eg[:, off:off + w],
                   start=(ck == 0), stop=(ck == nch - 1))

        def group(grp, gi):
            # grp: list of (h, ck, off, w); off in {0, 512}, slot widths
            # non-increasing so the ACT covers [0, off2+w2) with no junk when
            # w1==512 and a (w1-w2)-wide hole otherwise.
            def f():
                sg = psA.tile([128, 2 * SC], F32, tag="sg", name=f"sg{sqb}_{gi}")
                eg = expp.tile([128, 2 * SC], BF16, tag="exp", name=f"eg{sqb}_{gi}")
                for h, ck, off, w in grp:
                    qo = qoff(ck)
                    if h < 2:
                        b0 = h * 64
                        MM(sg[:, off:off + w],
                           kT[ck // CPB][b0:b0 + 64, (ck % CPB) * 128:(ck % CPB + 1) * 128],
                           qT[sqb][b0:b0 + 64, qo:qo + w], start=True, stop=True)
                    else:
                        rh = (ck % 2) * 64
                        MM(sg[:, off:off + w],
                           q2k2[ck // CPB][rh:rh + 64,
                                           SC + (ck % CPB // 2) * 128: SC + (ck % CPB // 2 + 1) * 128],
                           q2k2[sqb][rh:rh + 64, qo:qo + w], start=True, stop=True)
                EXP = mybir.ActivationFunctionType.Exp
                if len(grp) == 1:
                    w1 = grp[0][3]
                    nc.scalar.activation(eg[:, 0:w1], sg[:, 0:w1], EXP, scale=0.125)
                else:
                    w1, w2 = grp[0][3], grp[1][3]
                    if w1 == w2:
                        e3 = eg[:].rearrange("p (c n) -> p c n", c=2)[:, :, 0:w1]
                        s3 = sg[:].rearrange("p (c n) -> p c n", c=2)[:, :, 0:w1]
                        nc.scalar.activation(e3, s3, EXP, scale=0.125)
                    elif w1 == SC:
                        nc.scalar.activation(eg[:, 0:SC + w2], sg[:, 0:SC + w2],
                                             EXP, scale=0.125)
                    else:
                        e3 = eg[:].rearrange("p (c n) -> p c n", c=2)[:, :, 0:w1]
                        s3 = sg[:].rearrange("p (c n) -> p c n", c=2)[:, :, 0:w1]
                        nc.scalar.activation(e3, s3, EXP, scale=0.125)
                pend.append((eg, grp))
                if len(pend) > 2:
                    emit_pv()
            return f

        def flush_pv():
            while pend:
                emit_pv()

        def norm(h):
            def f():
                # reciprocal_approx_fast is a bitwise-seed op and misreads
                # PSUM; bounce the denominator row through SBUF first
                dn = smp.tile([1, SC], F32, tag="dn", name=f"dn{h}_{sqb}")
                nc.vector.tensor_copy(dn[:], cps[h][HD:HD + 1, :])
                rec = smp.tile([1, SC], F32, tag="rec", name=f"rec{h}_{sqb}")
                nc.vector.reciprocal_approx_fast(rec[:], dn[:])
                bc = smp.tile([64, SC], F32, tag="bc", name=f"bc{h}_{sqb}")
                nc.gpsimd.partition_broadcast(bc[:], rec[:])
                if h == 0:
                    nc.vector.tensor_mul(ctxAB[sqb][0:64, :], cps[h][0:HD, :], bc[:])
                elif h == 1:
                    nc.vector.tensor_mul(ctxAB[sqb][64:128, :], cps[h][0:HD, :], bc[:])
                else:
                    nc.vector.tensor_mul(ctxC[sqb][:], cps[h][0:HD, :], bc[:])
            return f

        thunks = []
        for hlist in ((0, 1), (2,)):
            slots = [(h, ck) for ck in range(nch) for h in hlist]
            thunks.append(alloc(hlist))
            for gi, g0 in enumerate(range(0, len(slots), 2)):
                grp = [(h, ck, si * SC, width(ck))
                       for si, (h, ck) in enumerate(slots[g0:g0 + 2])]
                thunks.append(group(grp, f"{hlist[0]}_{gi}"))
            thunks.append(flush_pv)
            for h in hlist:
                thunks.append(norm(h))
        return thunks

    # ---- interleaved emission ----
    for piece in proj_pieces(0):
        piece()
    for sqb in range(NQB):
        groups = attention_thunks(sqb)
        extras = []
        if sqb + 1 < NSC:
            extras += proj_pieces(sqb + 1)
        if sqb >= 1:
            for j in range(CPB):
                extras += outproj_pieces((sqb - 1) * CPB + j)
        n, k = len(groups), len(extras)
        ei = 0
        for i, g in enumerate(groups):
            g()
            due = (i + 1) * k // n
            while ei < due:
                extras[ei]()
                ei += 1
    for j in range(CPB):
        for piece in outproj_pieces((NQB - 1) * CPB + j):
            piece()

    for p in (psB, psA, ostp, smp, expp, ctxp, vp, q2p, kTp, qTp, xtp, constp):
        p.release()


def _build():
    if "nc" in _CACHE:
        return _CACHE["nc"]
    nc = bacc.Bacc("TRN2", target_bir_lowering=False, debug=False, num_devices=N_CORES)
    xT = nc.dram_tensor("xT", [D, S], BF16, kind="ExternalInput").ap()
    wqk = nc.dram_tensor("wqk", [D, 384], BF16, kind="ExternalInput").ap()
    wv = nc.dram_tensor("wv", [D, 192], BF16, kind="ExternalInput").ap()
    wo = nc.dram_tensor("wo", [HPC * HD, D], BF16, kind="ExternalInput").ap()
    out = nc.dram_tensor("out", [S, D], F32, kind="ExternalOutput").ap()
    with tile.TileContext(nc) as tc:
        _emit(nc, tc, (xT, wqk, wv, wo), out)
    nc.compile()
    _CACHE["nc"] = nc
    return nc


def _in_maps(x, w_qkv, w_out):
    import ml_dtypes
    xTs = [np.ascontiguousarray(x[b].T).astype(ml_dtypes.bfloat16) for b in range(B)]
    maps = []
    for c in range(N_CORES):
        b = c // 4
        h0 = (c % 4) * HPC
        cols = lambda base, h: w_qkv[:, base + (h0 + h) * HD: base + (h0 + h + 1) * HD]
        wqk = np.ascontiguousarray(np.concatenate(
            [cols(0, 0), cols(0, 1),            # m0: qA | qB
             cols(0, 2), cols(D, 2),            # m1: qC | kC
             cols(D, 0), cols(D, 1)], axis=1)).astype(ml_dtypes.bfloat16)
        wv = np.ascontiguousarray(np.concatenate(
            [cols(2 * D, 0), cols(2 * D, 1), cols(2 * D, 2)],
            axis=1)).astype(ml_dtypes.bfloat16)
        wo = np.ascontiguousarray(
            w_out[h0 * HD:(h0 + HPC) * HD, :]).astype(ml_dtypes.bfloat16)
        maps.append({"xT": xTs[b], "wqk": wqk, "wv": wv, "wo": wo})
    return maps


def run_sharded(x, w_qkv, w_out, **spmd_kwargs):
    nc = _build()
    res = run_bass_kernel_spmd(nc, _in_maps(x, w_qkv, w_out),
                               list(range(N_CORES)), **spmd_kwargs)
    outs = [res.results[c]["out"] for c in range(N_CORES)]
    y = np.empty((B, S, D), np.float32)
    for b in range(B):
        y[b] = outs[4 * b] + outs[4 * b + 1] + outs[4 * b + 2] + outs[4 * b + 3]
    return y, res


def kernel(x, w_qkv, w_out):
    x = np.asarray(x, dtype=np.float32)
    w_qkv = np.asarray(w_qkv, dtype=np.float32)
    w_out = np.asarray(w_out, dtype=np.float32)
    y, _ = run_sharded(x, w_qkv, w_out)
    return y

